# revision 9
# baseline (speedup 1.0000x reference)
"""Trainium2 Bass kernel for BioSignalAnomalyDetector (6-layer dense transformer).

Sharding: data-parallel over batch B=16 -> 2 batch elements per core on 8
NeuronCores.  No collectives; each core runs the full model on its shard.

On-chip dataflow (per core):
  - residual stream h kept token-major [token, D] in SBUF (LayerNorm natural)
  - h^T produced via PE transposes (feature-major) feeds all matmuls
  - attention computes scores^T = K^T.T @ Q^T per (head, k-tile), applies
    exp on the PSUM->SBUF eviction (scale fuses 1/sqrt(dk)), and defers
    softmax normalization: V gets a ones-column so the second matmul yields
    both ctx^T and the softmax denominators; normalization is a rank-1
    PE broadcast + one DVE multiply.
"""
import os
import sys

for _p in ("/opt/trn_rl_repo", "/root/.axon_site/_ro/trn_rl_repo"):
    if os.path.isdir(_p) and _p not in sys.path:
        sys.path.insert(0, _p)

import numpy as np

import concourse.bass as bass
import concourse.tile as tile
from concourse import bacc, mybir
from concourse.masks import make_identity

F32 = mybir.dt.float32
AF = mybir.ActivationFunctionType
ALU = mybir.AluOpType

# model dims (fixed by the problem)
B, S, IN, D, H, FF, L = 16, 1000, 32, 512, 8, 2048, 6
DK = D // H          # 64
EPS = 1e-5
NCORES = 8
BL = B // NCORES     # batch elements per core
ND = D // 128        # feature partition-tiles (4)
NF = FF // 128       # ffn partition-tiles (16)


def _ttiles(s):
    """token tiles of <=128 covering s tokens: [(t0, tsz), ...]"""
    out = []
    t = 0
    while t < s:
        out.append((t, min(128, s - t)))
        t += 128
    return out


def _chunks(s, ch):
    out = []
    t = 0
    while t < s:
        out.append((t, min(ch, s - t)))
        t += ch
    return out


def build_kernel(nc, cfg):
    """Emit the full model. cfg: dict with S, L, BL (for small-scale testing)."""
    s, nl, bl = cfg["S"], cfg["L"], cfg["BL"]
    CH = cfg.get("CH", 500)        # token chunk for feature-major free dims
    tt = _ttiles(s)                # per-b token tiles
    nt = len(tt)
    T = bl * s

    dt = F32

    # ---- DRAM I/O ----------------------------------------------------------
    def din(name, shape):
        return nc.dram_tensor(name, list(shape), dt, kind="ExternalInput").ap()

    x = din("x", (bl, s, IN))
    pe_bin = din("pe_bin", (s, D))            # pos-encoding + b_in (host-prepped)
    Win = din("Win", (IN, D))
    Wq = din("Wq", (nl, D, D)); bq = din("bq", (nl, D))
    Wk = din("Wk", (nl, D, D)); bk = din("bk", (nl, D))
    Wv = din("Wv", (nl, D, D)); bv = din("bv", (nl, D))
    Wo = din("Wo", (nl, D, D)); bo = din("bo", (nl, D))
    W1 = din("W1", (nl, D, FF)); b1 = din("b1", (nl, FF))
    W2 = din("W2", (nl, FF, D)); b2 = din("b2", (nl, D))
    ln1_g = din("ln1_g", (nl, D)); ln1_b = din("ln1_b", (nl, D))
    ln2_g = din("ln2_g", (nl, D)); ln2_b = din("ln2_b", (nl, D))
    lnf_g = din("lnf_g", (D,)); lnf_b = din("lnf_b", (D,))
    Wf1 = din("Wf1", (D, D // 2)); bf1 = din("bf1", (D // 2,))
    Wf2 = din("Wf2", (D // 2, D // 4)); bf2 = din("bf2", (D // 4,))
    Wr1 = din("Wr1", (D // 4, D // 2)); br1 = din("br1", (D // 2,))
    Wr2 = din("Wr2", (D // 2, D)); br2 = din("br2", (D,))
    Wr3 = din("Wr3", (D, IN)); br3 = din("br3", (IN,))
    Wa1 = din("Wa1", (D // 4, D // 8)); ba1 = din("ba1", (D // 8,))
    Wa2 = din("Wa2", (D // 8, 1)); ba2 = din("ba2", (1,))

    recon = nc.dram_tensor("recon", [bl, s, IN], dt, kind="ExternalOutput").ap()
    anomaly = nc.dram_tensor("anomaly", [bl, s, 1], dt, kind="ExternalOutput").ap()

    def bcast_pp(w_1d):
        """DRAM [n] -> AP [[0,128], [1,n]] replicating across partitions."""
        a = w_1d
        return bass.AP(tensor=a.tensor, offset=a.offset, ap=[[0, 128]] + a.ap)

    with tile.TileContext(nc) as tc:
        with (
            tc.tile_pool(name="const", bufs=1) as const,
            tc.tile_pool(name="resid", bufs=1) as resid,
            tc.tile_pool(name="hTp", bufs=1) as hTp,
        ):
            ident = const.tile([128, 128], dt)
            make_identity(nc, ident[:])
            ones128 = const.tile([1, 128], dt)
            nc.any.memset(ones128[:], 1.0)
            eps_t = const.tile([128, 1], dt)
            nc.any.memset(eps_t[:], EPS)

            # residual stream, token-major, per batch element
            h_sb = [resid.tile([128, nt, D], dt, tag=f"h{b}", name=f"h{b}") for b in range(bl)]
            # feature-major copy of the residual stream (both b stacked)
            hT = hTp.tile([128, ND, T], dt)

            # ---------------- input projection ------------------------------
            with (
                tc.tile_pool(name="inp", bufs=3) as inp,
                tc.tile_pool(name="inps", bufs=2, space="PSUM") as inps,
                tc.tile_pool(name="win", bufs=1) as winp,
            ):
                win_sb = winp.tile([IN, D], dt)
                nc.sync.dma_start(win_sb[:], Win)
                for b in range(bl):
                    for it, (t0, tsz) in enumerate(tt):
                        x_sb = inp.tile([128, IN], dt, tag="x")
                        nc.sync.dma_start(x_sb[:tsz], x[b, t0:t0 + tsz, :])
                        xT_ps = inps.tile([IN, 128], dt, tag="xT")
                        nc.tensor.transpose(xT_ps[:, :tsz], x_sb[:tsz, :IN], ident[:tsz, :tsz])
                        xT_sb = inp.tile([IN, 128], dt, tag="xTs")
                        nc.any.tensor_copy(xT_sb[:, :tsz], xT_ps[:, :tsz])
                        h_ps = inps.tile([128, D], dt, tag="h0")
                        nc.tensor.matmul(h_ps[:tsz], xT_sb[:, :tsz], win_sb[:],
                                         start=True, stop=True)
                        pe_sb = inp.tile([128, D], dt, tag="pe")
                        nc.sync.dma_start(pe_sb[:tsz], pe_bin[t0:t0 + tsz, :])
                        nc.vector.tensor_add(h_sb[b][:tsz, it, :], h_ps[:tsz], pe_sb[:tsz])

            def transpose_h_into_hT():
                with tc.tile_pool(name="trps", bufs=4, space="PSUM") as trps:
                    for b in range(bl):
                        for it, (t0, tsz) in enumerate(tt):
                            for c in range(ND):
                                tp = trps.tile([128, 128], dt, tag="tr")
                                nc.tensor.transpose(
                                    tp[:, :tsz],
                                    h_sb[b][:tsz, it, c * 128:(c + 1) * 128],
                                    ident[:tsz, :tsz])
                                g0 = b * s + t0
                                nc.any.tensor_copy(hT[:, c, g0:g0 + tsz], tp[:, :tsz])

            def layernorm_tile(pool, out_ap, in0_ps, res_ap, badd_bc, g_bc, b_bc, tsz):
                """out = LN(in0_ps + res_ap + badd_bc) * g_bc + b_bc  (token-major)."""
                if in0_ps is None:
                    t1 = pool.tile([128, D], dt, tag="ln_t1")
                    nc.vector.tensor_copy(t1[:tsz], res_ap)
                else:
                    t1 = pool.tile([128, D], dt, tag="ln_t1")
                    nc.vector.tensor_add(t1[:tsz], in0_ps, res_ap)
                if badd_bc is not None:
                    nc.vector.tensor_add(t1[:tsz], t1[:tsz], badd_bc[:tsz])
                st = pool.tile([128, 6], dt, tag="ln_st")
                nc.vector.bn_stats(st[:tsz], t1[:tsz])
                mv = pool.tile([128, 2], dt, tag="ln_mv")
                nc.vector.bn_aggr(mv[:tsz], st[:tsz])
                sd = pool.tile([128, 1], dt, tag="ln_sd")
                nc.scalar.activation(sd[:tsz], mv[:tsz, 1:2], AF.Sqrt, bias=eps_t[:tsz])
                nc.vector.reciprocal(sd[:tsz], sd[:tsz])
                t2 = pool.tile([128, D], dt, tag="ln_t2")
                nc.vector.tensor_scalar(t2[:tsz], t1[:tsz], mv[:tsz, 0:1], sd[:tsz],
                                        op0=ALU.subtract, op1=ALU.mult)
                nc.vector.tensor_mul(t2[:tsz], t2[:tsz], g_bc[:tsz])
                nc.vector.tensor_add(out_ap, t2[:tsz], b_bc[:tsz])

            # ---------------- encoder layers --------------------------------
            for layer in range(nl):
                transpose_h_into_hT()

                # ---- QKV + attention ----
                with (
                    tc.tile_pool(name="wqkv", bufs=1) as wqkv,
                    tc.tile_pool(name="qkv", bufs=1) as qkv,
                    tc.tile_pool(name="ctxp", bufs=1) as ctxp,
                    tc.tile_pool(name="att", bufs=3) as att,
                    tc.tile_pool(name="aps", bufs=1, space="PSUM") as aps,
                ):
                    wq_sb = wqkv.tile([128, ND, D], dt, tag="wq")
                    wk_sb = wqkv.tile([128, ND, D], dt, tag="wk")
                    wv_sb = wqkv.tile([128, ND, D], dt, tag="wv")
                    nc.sync.dma_start(wq_sb[:], Wq[layer].rearrange("(c p) n -> p c n", p=128))
                    nc.sync.dma_start(wk_sb[:], Wk[layer].rearrange("(c p) n -> p c n", p=128))
                    nc.sync.dma_start(wv_sb[:], Wv[layer].rearrange("(c p) n -> p c n", p=128))
                    bq_sb = wqkv.tile([128, ND], dt, tag="bq")
                    bk_sb = wqkv.tile([128, ND], dt, tag="bk")
                    nc.sync.dma_start(bq_sb[:], bq[layer].rearrange("(c p) -> p c", p=128))
                    nc.sync.dma_start(bk_sb[:], bk[layer].rearrange("(c p) -> p c", p=128))
                    bv_bc = wqkv.tile([128, D], dt, tag="bv")
                    nc.gpsimd.dma_start(bv_bc[:], bcast_pp(bv[layer]))

                    wo_sb = wqkv.tile([128, ND, D], dt, tag="wo")
                    nc.sync.dma_start(wo_sb[:], Wo[layer].rearrange("(c p) n -> p c n", p=128))
                    bo_bc = wqkv.tile([128, D], dt, tag="bo")
                    nc.gpsimd.dma_start(bo_bc[:], bcast_pp(bo[layer]))
                    g1_bc = wqkv.tile([128, D], dt, tag="g1")
                    b1_bc = wqkv.tile([128, D], dt, tag="b1")
                    nc.gpsimd.dma_start(g1_bc[:], bcast_pp(ln1_g[layer]))
                    nc.gpsimd.dma_start(b1_bc[:], bcast_pp(ln1_b[layer]))

                    HG = 2               # head groups (halves qT/kT SBUF)
                    HPG = H // HG        # heads per group
                    NOC = ND // HG       # dout 128-chunks per group
                    for b in range(bl):
                        ctxT = ctxp.tile([128, ND, s], dt, tag="ctxT", name="ctxT")
                        # V token-major with ones column: [t, kt, h, DK+1]
                        vt = qkv.tile([128, nt, H, DK + 1], dt, tag="vt")
                        nc.any.memset(vt[:, :, :, DK:DK + 1], 1.0)
                        for it, (t0, tsz) in enumerate(tt):
                            ps = aps.tile([128, D], dt, tag="big", bufs=3)
                            for c in range(ND):
                                nc.tensor.matmul(
                                    ps[:tsz], hT[:, c, b * s + t0:b * s + t0 + tsz],
                                    wv_sb[:, c, :], start=(c == 0), stop=(c == ND - 1))
                            nc.vector.tensor_add(
                                vt[:tsz, it, :, 0:DK],
                                ps[:tsz].rearrange("p (h e) -> p h e", h=H),
                                bv_bc[:tsz].rearrange("p (h e) -> p h e", h=H))

                        for hg in range(HG):
                            # Q^T, K^T feature-major for this head group
                            qT = qkv.tile([128, NOC, s], dt, tag="qT")
                            kT = qkv.tile([128, NOC, s], dt, tag="kT")
                            for dst, w_sb, b_sb in ((qT, wq_sb, bq_sb), (kT, wk_sb, bk_sb)):
                                for oc in range(NOC):
                                    goc = hg * NOC + oc
                                    for q0, qsz in _chunks(s, CH):
                                        ps = aps.tile([128, CH], dt, tag="big", bufs=3)
                                        for c in range(ND):
                                            nc.tensor.matmul(
                                                ps[:, :qsz],
                                                w_sb[:, c, goc * 128:(goc + 1) * 128],
                                                hT[:, c, b * s + q0:b * s + q0 + qsz],
                                                start=(c == 0), stop=(c == ND - 1))
                                        nc.vector.tensor_scalar_add(
                                            dst[:, oc, q0:q0 + qsz], ps[:, :qsz],
                                            b_sb[:, goc:goc + 1])

                            for hl in range(HPG):
                                hd = hg * HPG + hl
                                hc, ho = hl // 2, (hl % 2) * DK
                                for q0, qsz in _chunks(s, CH):
                                    cps = aps.tile([DK + 1, CH], dt, tag="ctx", bufs=2)
                                    for it, (t0, tsz) in enumerate(tt):
                                        sps = aps.tile([128, CH], dt, tag="big", bufs=3)
                                        nc.tensor.matmul(
                                            sps[:tsz, :qsz],
                                            kT[ho:ho + DK, hc, t0:t0 + tsz],
                                            qT[ho:ho + DK, hc, q0:q0 + qsz],
                                            start=True, stop=True)
                                        ex = att.tile([128, CH], dt, tag="exp")
                                        nc.scalar.activation(ex[:tsz, :qsz], sps[:tsz, :qsz],
                                                             AF.Exp, scale=1.0 / np.sqrt(DK))
                                        nc.tensor.matmul(
                                            cps[:, :qsz], vt[:tsz, it, hd, :], ex[:tsz, :qsz],
                                            start=(it == 0), stop=(it == nt - 1))
                                    # normalize: rows 0..DK-1 / row DK
                                    rc = att.tile([1, CH], dt, tag="recip", bufs=2)
                                    nc.vector.reciprocal(rc[:, :qsz], cps[DK:DK + 1, :qsz])
                                    bps = aps.tile([DK, CH], dt, tag="bc", bufs=2)
                                    nc.tensor.matmul(bps[:, :qsz], ones128[:, :DK], rc[:, :qsz],
                                                     start=True, stop=True)
                                    bsb = att.tile([DK, CH], dt, tag="bcs", bufs=2)
                                    nc.scalar.activation(bsb[:, :qsz], bps[:, :qsz], AF.Copy)
                                    nc.vector.tensor_mul(
                                        ctxT[ho:ho + DK, hg * NOC + hc, q0:q0 + qsz],
                                        cps[0:DK, :qsz], bsb[:, :qsz])

                        # ---- attn_out = ctx @ Wo; h = LN1(h + attn_out + bo) ----
                        for it, (t0, tsz) in enumerate(tt):
                            ps = aps.tile([128, D], dt, tag="big", bufs=3)
                            for c in range(ND):
                                nc.tensor.matmul(
                                    ps[:tsz], ctxT[:, c, t0:t0 + tsz],
                                    wo_sb[:, c, :], start=(c == 0), stop=(c == ND - 1))
                            layernorm_tile(att, h_sb[b][:tsz, it, :], ps[:tsz],
                                           h_sb[b][:tsz, it, :], bo_bc, g1_bc, b1_bc, tsz)

                transpose_h_into_hT()

                # ---- FFN ----
                with (
                    tc.tile_pool(name="wffn", bufs=1) as wffn,
                    tc.tile_pool(name="ffp", bufs=2) as ffp,
                    tc.tile_pool(name="fft", bufs=3) as fft,
                    tc.tile_pool(name="ffn_ps", bufs=3, space="PSUM") as ffn_ps,
                    tc.tile_pool(name="ffn_ps2", bufs=2, space="PSUM") as ffn_ps2,
                ):
                    w1_sb = wffn.tile([128, ND, FF], dt, tag="w1")
                    nc.sync.dma_start(w1_sb[:], W1[layer].rearrange("(c p) n -> p c n", p=128))
                    b1_sb = wffn.tile([128, NF], dt, tag="b1f")
                    nc.sync.dma_start(b1_sb[:], b1[layer].rearrange("(c p) -> p c", p=128))
                    w2_sb = wffn.tile([128, NF, D], dt, tag="w2")
                    nc.sync.dma_start(w2_sb[:], W2[layer].rearrange("(c p) n -> p c n", p=128))
                    b2_bc = wffn.tile([128, D], dt, tag="b2f")
                    nc.gpsimd.dma_start(b2_bc[:], bcast_pp(b2[layer]))
                    g2_bc = wffn.tile([128, D], dt, tag="g2")
                    bb2_bc = wffn.tile([128, D], dt, tag="bb2")
                    nc.gpsimd.dma_start(g2_bc[:], bcast_pp(ln2_g[layer]))
                    nc.gpsimd.dma_start(bb2_bc[:], bcast_pp(ln2_b[layer]))

                    # chunks of two token-tiles, aligned with the h tiling
                    for b in range(bl):
                        for ig in range(0, nt, 2):
                            pair = tt[ig:ig + 2]
                            c0 = pair[0][0]
                            csz = sum(p[1] for p in pair)
                            ffT = ffp.tile([128, NF, 256], dt, tag="ffT")
                            for fc in range(NF):
                                ps = ffn_ps.tile([128, 256], dt, tag="ff1")
                                for c in range(ND):
                                    nc.tensor.matmul(
                                        ps[:, :csz],
                                        w1_sb[:, c, fc * 128:(fc + 1) * 128],
                                        hT[:, c, b * s + c0:b * s + c0 + csz],
                                        start=(c == 0), stop=(c == ND - 1))
                                nc.scalar.activation(ffT[:, fc, :csz], ps[:, :csz],
                                                     AF.Relu, bias=b1_sb[:, fc:fc + 1])
                            for it, (t0, tsz) in enumerate(pair):
                                off = t0 - c0
                                ps2 = ffn_ps2.tile([128, D], dt, tag="ff2")
                                for kc in range(NF):
                                    nc.tensor.matmul(
                                        ps2[:tsz], ffT[:, kc, off:off + tsz],
                                        w2_sb[:, kc, :],
                                        start=(kc == 0), stop=(kc == NF - 1))
                                iti = ig + it
                                layernorm_tile(fft, h_sb[b][:tsz, iti, :], ps2[:tsz],
                                               h_sb[b][:tsz, iti, :], b2_bc,
                                               g2_bc, bb2_bc, tsz)

            # ---------------- final LN + heads ------------------------------
            with (
                tc.tile_pool(name="lnf", bufs=3) as lnf_pool,
                tc.tile_pool(name="lnfw", bufs=1) as lnfw,
            ):
                gf_bc = lnfw.tile([128, D], dt, tag="gf")
                bf_bc = lnfw.tile([128, D], dt, tag="bf")
                nc.gpsimd.dma_start(gf_bc[:], bcast_pp(lnf_g))
                nc.gpsimd.dma_start(bf_bc[:], bcast_pp(lnf_b))
                for b in range(bl):
                    for it, (t0, tsz) in enumerate(tt):
                        layernorm_tile(lnf_pool, h_sb[b][:tsz, it, :], None,
                                       h_sb[b][:tsz, it, :], None, gf_bc, bf_bc, tsz)
            transpose_h_into_hT()

            with (
                tc.tile_pool(name="hw", bufs=1) as hw,
                tc.tile_pool(name="hact", bufs=3) as hact,
                tc.tile_pool(name="h_ps", bufs=4, space="PSUM") as h_ps,
            ):
                wf1_sb = hw.tile([128, ND, D // 2], dt, tag="wf1")
                nc.sync.dma_start(wf1_sb[:], Wf1.rearrange("(c p) n -> p c n", p=128))
                bf1_sb = hw.tile([128, 2], dt, tag="bf1")
                nc.sync.dma_start(bf1_sb[:], bf1.rearrange("(c p) -> p c", p=128))
                wf2_sb = hw.tile([128, 2, D // 4], dt, tag="wf2")
                nc.sync.dma_start(wf2_sb[:], Wf2.rearrange("(c p) n -> p c n", p=128))
                bf2_sb = hw.tile([128, 1], dt, tag="bf2")
                nc.sync.dma_start(bf2_sb[:], bf2.rearrange("(c p) -> p c", p=128))
                wr1_sb = hw.tile([128, D // 2], dt, tag="wr1")
                nc.sync.dma_start(wr1_sb[:], Wr1)
                br1_sb = hw.tile([128, 2], dt, tag="br1")
                nc.sync.dma_start(br1_sb[:], br1.rearrange("(c p) -> p c", p=128))
                wr2_sb = hw.tile([128, 2, D], dt, tag="wr2")
                nc.sync.dma_start(wr2_sb[:], Wr2.rearrange("(c p) n -> p c n", p=128))
                br2_sb = hw.tile([128, ND], dt, tag="br2")
                nc.sync.dma_start(br2_sb[:], br2.rearrange("(c p) -> p c", p=128))
                wr3_sb = hw.tile([128, ND, IN], dt, tag="wr3")
                nc.sync.dma_start(wr3_sb[:], Wr3.rearrange("(c p) n -> p c n", p=128))
                br3_sb = hw.tile([IN, 1], dt, tag="br3")
                nc.sync.dma_start(br3_sb[:], br3.rearrange("(p o) -> p o", o=1))
                wa1_sb = hw.tile([128, D // 8], dt, tag="wa1")
                nc.sync.dma_start(wa1_sb[:], Wa1)
                ba1_sb = hw.tile([D // 8, 1], dt, tag="ba1")
                nc.sync.dma_start(ba1_sb[:], ba1.rearrange("(p o) -> p o", o=1))
                wa2_sb = hw.tile([D // 8, 1], dt, tag="wa2")
                nc.sync.dma_start(wa2_sb[:], Wa2)
                ba2_sb = hw.tile([1, 1], dt, tag="ba2")
                nc.sync.dma_start(ba2_sb[:], ba2.rearrange("(p o) -> p o", o=1))

                for b in range(bl):
                    for q0, qsz in _chunks(s, CH):
                        g0 = b * s + q0
                        f1T = hact.tile([128, 2, CH], dt, tag="f1T")
                        for oc in range(2):
                            ps = h_ps.tile([128, CH], dt, tag="hps")
                            for c in range(ND):
                                nc.tensor.matmul(
                                    ps[:, :qsz], wf1_sb[:, c, oc * 128:(oc + 1) * 128],
                                    hT[:, c, g0:g0 + qsz],
                                    start=(c == 0), stop=(c == ND - 1))
                            nc.scalar.activation(f1T[:, oc, :qsz], ps[:, :qsz],
                                                 AF.Relu, bias=bf1_sb[:, oc:oc + 1])
                        featT = hact.tile([128, CH], dt, tag="featT")
                        ps = h_ps.tile([128, CH], dt, tag="hps")
                        for c in range(2):
                            nc.tensor.matmul(ps[:, :qsz], wf2_sb[:, c, :], f1T[:, c, :qsz],
                                             start=(c == 0), stop=(c == 1))
                        nc.vector.tensor_scalar_add(featT[:, :qsz], ps[:, :qsz], bf2_sb[:])
                        # recon head
                        r1T = hact.tile([128, 2, CH], dt, tag="r1T")
                        for oc in range(2):
                            ps = h_ps.tile([128, CH], dt, tag="hps")
                            nc.tensor.matmul(ps[:, :qsz], wr1_sb[:, oc * 128:(oc + 1) * 128],
                                             featT[:, :qsz], start=True, stop=True)
                            nc.scalar.activation(r1T[:, oc, :qsz], ps[:, :qsz],
                                                 AF.Relu, bias=br1_sb[:, oc:oc + 1])
                        r2T = hact.tile([128, ND, CH], dt, tag="r2T")
                        for oc in range(ND):
                            ps = h_ps.tile([128, CH], dt, tag="hps")
                            for c in range(2):
                                nc.tensor.matmul(
                                    ps[:, :qsz], wr2_sb[:, c, oc * 128:(oc + 1) * 128],
                                    r1T[:, c, :qsz], start=(c == 0), stop=(c == 1))
                            nc.scalar.activation(r2T[:, oc, :qsz], ps[:, :qsz],
                                                 AF.Relu, bias=br2_sb[:, oc:oc + 1])
                        rT = hact.tile([IN, CH], dt, tag="rT")
                        ps = h_ps.tile([IN, CH], dt, tag="hps")
                        for c in range(ND):
                            nc.tensor.matmul(ps[:, :qsz], wr3_sb[:, c, :], r2T[:, c, :qsz],
                                             start=(c == 0), stop=(c == ND - 1))
                        nc.vector.tensor_scalar_add(rT[:, :qsz], ps[:, :qsz], br3_sb[:])
                        # transpose recon chunks back to token-major and store
                        for st0, ssz in _chunks(qsz, 125):
                            tp = h_ps.tile([128, IN], dt, tag="hps")
                            nc.tensor.transpose(tp[:ssz, :IN], rT[:, st0:st0 + ssz],
                                                ident[:IN, :IN])
                            ro = hact.tile([128, IN], dt, tag="ro")
                            nc.any.tensor_copy(ro[:ssz], tp[:ssz, :IN])
                            nc.sync.dma_start(recon[b, q0 + st0:q0 + st0 + ssz, :], ro[:ssz])
                        # anomaly head
                        a1T = hact.tile([D // 8, CH], dt, tag="a1T")
                        ps = h_ps.tile([D // 8, CH], dt, tag="hps")
                        nc.tensor.matmul(ps[:, :qsz], wa1_sb[:], featT[:, :qsz],
                                         start=True, stop=True)
                        nc.scalar.activation(a1T[:, :qsz], ps[:, :qsz], AF.Relu,
                                             bias=ba1_sb[:])
                        ps2 = h_ps.tile([1, CH], dt, tag="hps")
                        nc.tensor.matmul(ps2[:, :qsz], wa2_sb[:], a1T[:, :qsz],
                                         start=True, stop=True)
                        an = hact.tile([1, CH], dt, tag="an")
                        nc.scalar.activation(an[:, :qsz], ps2[:, :qsz], AF.Sigmoid,
                                             bias=ba2_sb[:])
                        nc.sync.dma_start(anomaly[b, q0:q0 + qsz, 0:1], an[:, :qsz])

    return nc


# ---------------------------------------------------------------------------
_CACHE = {}


def _pos_encoding_np(seq_len, d_model):
    pos = np.arange(seq_len, dtype=np.float32)[:, None]
    div = np.exp(np.arange(0, d_model, 2, dtype=np.float32)
                 * (-np.log(10000.0) / d_model))
    pe = np.zeros((seq_len, d_model), np.float32)
    pe[:, 0::2] = np.sin(pos * div)
    pe[:, 1::2] = np.cos(pos * div)
    return pe


def _get_compiled():
    if "nc" not in _CACHE:
        nc = bacc.Bacc("TRN2", target_bir_lowering=False, debug=False)
        build_kernel(nc, {"S": S, "L": L, "BL": BL})
        nc.compile()
        _CACHE["nc"] = nc
    return _CACHE["nc"]


def _run(inputs, trace=False, trace_kwargs=None):
    from concourse.bass_utils import run_bass_kernel_spmd

    nc = _get_compiled()
    inp = {k: np.asarray(v, dtype=np.float32) for k, v in inputs.items()}
    pe_bin = _pos_encoding_np(S, D) + inp["b_in"][None, :]

    in_maps = []
    for core in range(NCORES):
        m = {"pe_bin": pe_bin}
        m["x"] = np.ascontiguousarray(inp["x"][core * BL:(core + 1) * BL])
        for k, v in inp.items():
            if k in ("x", "b_in"):
                continue
            m[k] = v
        in_maps.append(m)

    res = run_bass_kernel_spmd(nc, in_maps, list(range(NCORES)),
                               trace=trace, **(trace_kwargs or {}))
    recon = np.concatenate([r["recon"] for r in res.results], axis=0)
    anomaly = np.concatenate([r["anomaly"] for r in res.results], axis=0)
    return recon, anomaly, res


def kernel(**inputs):
    recon, anomaly, _ = _run(inputs)
    return recon, anomaly
